# revision 22
# baseline (speedup 1.0000x reference)
"""Trainium2 Bass kernel for a pre-norm transformer block (B=16,N=1024,C=768,H=12).

Strategy: pure data-parallel over batch, 2 batch elements per NeuronCore (8 cores),
no collectives. Activations are kept feature-major on device ([C, tokens]); the
host transposes x in / out (layout packing only). All matmuls run in bf16 with
fp32 PSUM accumulation; the residual stream stays fp32 end to end, so bf16 error
in the branches is suppressed by the 1e-5 LayerScale at the output.

LayerNorm runs in feature-major form: per-token sums over features are computed
on the TensorEngine with a ones-column matmul; per-token scale/shift rows are
broadcast across partitions with K=1 matmuls; per-feature gamma/beta become
per-partition tensor_scalar operands.

Attention computes transposed scores S^T[tk,tq] = K_h^T·Q_h directly (Q,K are
feature-major slices of the QKV output), exp on ScalarE (no max-subtract:
scores are O(1) here, softmax is shift-invariant), and folds the softmax
denominator into the AV matmul via a ones-column appended to V^T (V is produced
token-major by an activation-stationary matmul). Normalization multiplies by
the broadcast reciprocal of the accumulated denominator row.
"""

import numpy as np
import ml_dtypes
from contextlib import ExitStack

import concourse.bass as bass
import concourse.tile as tile
import concourse.mybir as mybir
from concourse.bass_utils import run_bass_kernel_spmd
from concourse.mybir import AluOpType as alu
from concourse.mybir import ActivationFunctionType as act

F32 = mybir.dt.float32
BF16 = mybir.dt.bfloat16
BF16_NP = ml_dtypes.bfloat16

B, N, C, H, HD, MLP = 16, 1024, 768, 12, 64, 3072
EPS = 1e-5
NCORES = 8
BPC = B // NCORES          # batch elems per core
T = BPC * N                # tokens per core (2048)
CK = 512                   # token chunk
NCH = T // CK              # 4 chunks per core
FT = C // 128              # 6 feature tiles
QKT = 12                   # q+k output 128-col tiles (1536 cols)
VT = MLP // 128            # 24 fc1 tiles
TKT = N // 128             # 8 key tiles per batch elem
NTQ = N // CK              # 2 query chunks per batch elem


def _patched_drain_and_barrier(self, tick_clock, wait_clock):
    # This walrus build rejects >2 sync waits on one Drain ("Too many sync
    # wait commands"); spread the end-of-kernel waits over single-wait NOPs.
    import bass_rust
    from concourse.vector_clock import ScopedClock

    drain_inst = self.nc.sync.drain()
    wait_clock.add_sem_waits(
        drain_inst.ins, ScopedClock({None: tick_clock.global_clock})
    )
    si = drain_inst.ins.sync_info
    waits = list(si.on_wait) if si is not None and si.on_wait else []
    if len(waits) > 1:
        si.on_wait = waits[:1]
        for w in waits[1:]:
            nop = self.nc.sync.nop(nofuse=True)
            nsi = nop.ins.sync_info
            if nsi is None:
                nop.ins.sync_info = bass_rust.SyncInfo(on_wait=[w], on_update=[])
            else:
                nsi.on_wait = [w]
    self.nc.all_engine_barrier()
    popped = self.nc._tile_sem_poison_stack.pop()
    assert popped is self._sem_poison
    self.nc.clear_and_free_semaphores(list(self.sems.allocated().values()))
    self.nc.all_engine_barrier()


tile.TileContext._drain_and_barrier = _patched_drain_and_barrier

_MAXW = 1  # this walrus build rejects multiple sync waits on one instruction


def _split_sync_waits(nc):
    """Walrus here caps per-instruction sync waits; move the excess onto
    same-engine NOPs inserted immediately before the offending instruction
    (engine program order makes this equivalent)."""
    import bass_rust

    nsplit = 0
    for bb in nc.m.functions[0].blocks:
        insts = bb.instructions
        i = 0
        while i < len(insts):
            inst = insts[i]
            si = inst.sync_info
            if si is not None and si.on_wait and len(si.on_wait) > _MAXW:
                waits = list(si.on_wait)
                si.on_wait = waits[:_MAXW]
                extra = waits[_MAXW:]
                pos = i
                for j in range(0, len(extra), _MAXW):
                    nop = mybir.InstNoOp(
                        name=f"{inst.name}_wsplit{j}",
                        engine=inst.engine,
                        bass_nofuse=True,
                        sync_info=bass_rust.SyncInfo(
                            on_wait=extra[j:j + _MAXW], on_update=[]),
                    )
                    insts.insert(pos, nop)
                    pos += 1
                    i += 1
                    nsplit += 1
            i += 1
    return nsplit


_CACHE = {}


def _emit_ln(nc, pools, xc, g_sb, b_sb, ft_count, y_out, ones_col, ones_row,
             eps_row):
    """LayerNorm over the feature (partition) axis of one token chunk.

    xc:    [128, ft_count, CK] f32 sbuf tile (feature-major chunk)
    y_out: [128, ft_count, CK] bf16 sbuf tile
    """
    rows, scratch, ps_rows, ps_bc = (
        pools["rows"], pools["scratch"], pools["ps_rows"], pools["ps_bc"]
    )
    ps_s = ps_rows.tile([1, CK], F32, tag="ssum")
    ps_q = ps_rows.tile([1, CK], F32, tag="sqsum")
    for ft in range(ft_count):
        nc.tensor.matmul(ps_s, lhsT=ones_col, rhs=xc[:, ft, :],
                         start=(ft == 0), stop=(ft == ft_count - 1))
    for ft in range(ft_count):
        xq = scratch.tile([128, CK], F32, tag="sq")
        nc.vector.tensor_mul(xq, xc[:, ft, :], xc[:, ft, :])
        nc.tensor.matmul(ps_q, lhsT=ones_col, rhs=xq,
                         start=(ft == 0), stop=(ft == ft_count - 1))
    inv_c = 1.0 / (ft_count * 128)
    mu = rows.tile([1, CK], F32, tag="mu")
    nc.vector.tensor_scalar_mul(mu, ps_s, inv_c)
    ex2 = rows.tile([1, CK], F32, tag="ex2")
    nc.vector.tensor_scalar_mul(ex2, ps_q, inv_c)
    mu2 = rows.tile([1, CK], F32, tag="mu2")
    nc.vector.tensor_mul(mu2, mu, mu)
    nc.vector.tensor_sub(ex2, ex2, mu2)            # ex2 <- var
    nc.scalar.activation(ex2, ex2, act.Sqrt, bias=eps_row)  # ex2 <- std
    rs = rows.tile([1, CK], F32, tag="rs")
    nc.vector.reciprocal(rs, ex2)
    nb = rows.tile([1, CK], F32, tag="nb")
    nc.vector.scalar_tensor_tensor(nb, in0=mu, scalar=-1.0, in1=rs,
                                   op0=alu.mult, op1=alu.mult)
    bc_a = ps_bc.tile([128, CK], F32, tag="bca")
    nc.tensor.matmul(bc_a, lhsT=ones_row, rhs=rs, start=True, stop=True)
    bc_b = ps_bc.tile([128, CK], F32, tag="bcb")
    nc.tensor.matmul(bc_b, lhsT=ones_row, rhs=nb, start=True, stop=True)
    for ft in range(ft_count):
        t1 = scratch.tile([128, CK], F32, tag="t1")
        nc.vector.tensor_tensor(t1, xc[:, ft, :], bc_a, alu.mult)
        t2 = scratch.tile([128, CK], F32, tag="t2")
        nc.vector.tensor_tensor(t2, t1, bc_b, alu.add)
        nc.vector.tensor_scalar(y_out[:, ft, :], t2,
                                scalar1=g_sb[:, ft:ft + 1],
                                scalar2=b_sb[:, ft:ft + 1],
                                op0=alu.mult, op1=alu.add)


def _build_program():
    if "nc" in _CACHE:
        return _CACHE["nc"]
    nc = bass.Bass()

    xT_d = nc.dram_tensor("xT", [FT, 128, T], F32, kind="ExternalInput")
    wqkv_d = nc.dram_tensor("wqkv", [FT, 128, 3 * C], BF16, kind="ExternalInput")
    wproj_d = nc.dram_tensor("wproj", [FT, 128, C], BF16, kind="ExternalInput")
    wfc1_d = nc.dram_tensor("wfc1", [FT, 128, MLP], BF16, kind="ExternalInput")
    wfc2_d = nc.dram_tensor("wfc2", [VT, 128, C], BF16, kind="ExternalInput")
    ln1g_d = nc.dram_tensor("ln1g", [128, FT], F32, kind="ExternalInput")
    ln1b_d = nc.dram_tensor("ln1b", [128, FT], F32, kind="ExternalInput")
    ln2g_d = nc.dram_tensor("ln2g", [128, FT], F32, kind="ExternalInput")
    ln2b_d = nc.dram_tensor("ln2b", [128, FT], F32, kind="ExternalInput")
    bls1_d = nc.dram_tensor("bls1", [128, FT], F32, kind="ExternalInput")
    bls2_d = nc.dram_tensor("bls2", [128, FT], F32, kind="ExternalInput")
    bfc1_d = nc.dram_tensor("bfc1", [128, VT], F32, kind="ExternalInput")
    outT_d = nc.dram_tensor("outT", [FT, 128, T], F32, kind="ExternalOutput")

    with tile.TileContext(nc) as tc, ExitStack() as ctx:
        const = ctx.enter_context(tc.tile_pool(name="const", bufs=1))
        params = ctx.enter_context(tc.tile_pool(name="params", bufs=1))
        rows = ctx.enter_context(tc.tile_pool(name="rows", bufs=2))
        scratch = ctx.enter_context(tc.tile_pool(name="scratch", bufs=2))

        ones_col = const.tile([128, 1], F32)
        nc.vector.memset(ones_col, 1.0)
        ones_row = const.tile([1, 128], F32)
        nc.vector.memset(ones_row, 1.0)
        eps_row = const.tile([1, 1], F32)
        nc.vector.memset(eps_row, EPS)

        ln1g = params.tile([128, FT], F32)
        nc.sync.dma_start(ln1g, ln1g_d[:, :])
        ln1b = params.tile([128, FT], F32)
        nc.sync.dma_start(ln1b, ln1b_d[:, :])
        ln2g = params.tile([128, FT], F32)
        nc.sync.dma_start(ln2g, ln2g_d[:, :])
        ln2b = params.tile([128, FT], F32)
        nc.sync.dma_start(ln2b, ln2b_d[:, :])
        bls1 = params.tile([128, FT], F32)
        nc.sync.dma_start(bls1, bls1_d[:, :])
        bls2 = params.tile([128, FT], F32)
        nc.sync.dma_start(bls2, bls2_d[:, :])
        bfc1 = params.tile([128, VT], F32)
        nc.sync.dma_start(bfc1, bfc1_d[:, :])

        qkv_stack = ExitStack()
        qkv_sb = qkv_stack.enter_context(tc.tile_pool(name="qkv_sb", bufs=1))
        q_t = qkv_sb.tile([128, FT, T], BF16, tag="q")
        k_t = qkv_sb.tile([128, FT, T], BF16, tag="k")
        v_t = qkv_sb.tile([128, T // 128, H, HD + 1], BF16, tag="v")
        nc.vector.memset(v_t[:, :, :, HD:HD + 1], 1.0)

        # ---------------- Stage A: LN1 + QKV ----------------
        with tc.tile_pool(name="wqkv_sb", bufs=1) as wq_pool, \
             tc.tile_pool(name="xa", bufs=2) as xa_pool, \
             tc.tile_pool(name="y1", bufs=2) as y1_pool, \
             tc.tile_pool(name="ps_rows_a", bufs=1, space="PSUM") as ps_rows_a, \
             tc.tile_pool(name="ps_bc_a", bufs=1, space="PSUM") as ps_bc_a, \
             tc.tile_pool(name="ps_mm_a", bufs=3, space="PSUM") as ps_mm:
            ln_pools = {"rows": rows, "scratch": scratch,
                        "ps_rows": ps_rows_a, "ps_bc": ps_bc_a}
            wqkv_t = wq_pool.tile([128, FT, 3 * C], BF16)
            for kt in range(FT):
                nc.sync.dma_start(wqkv_t[:, kt, :], wqkv_d[kt, :, :])

            for ch in range(NCH):
                c0 = ch * CK
                xc = xa_pool.tile([128, FT, CK], F32)
                for ft in range(FT):
                    nc.sync.dma_start(xc[:, ft, :], xT_d[ft, :, c0:c0 + CK])
                y1 = y1_pool.tile([128, FT, CK], BF16)
                _emit_ln(nc, ln_pools, xc, ln1g, ln1b, FT, y1, ones_col,
                         ones_row, eps_row)

                # Q,K: weight-stationary -> feature-major [1536, CK]
                for mt in range(QKT):
                    ps = ps_mm.tile([128, CK], F32, tag="mm")
                    for kt in range(FT):
                        nc.tensor.matmul(
                            ps,
                            lhsT=wqkv_t[:, kt, mt * 128:(mt + 1) * 128],
                            rhs=y1[:, kt, :],
                            start=(kt == 0), stop=(kt == FT - 1))
                    dst = q_t if mt < FT else k_t
                    nc.vector.tensor_copy(dst[:, mt % FT, c0:c0 + CK], ps)

                # V: activation-stationary -> token-major [CK, 768]
                for mtok in range(CK // 128):
                    gtok = ch * (CK // 128) + mtok
                    for nv in range(2):
                        ps = ps_mm.tile([128, CK], F32, tag="mm")
                        psv = ps[:, 0:384]
                        for kt in range(FT):
                            nc.tensor.matmul(
                                psv,
                                lhsT=y1[:, kt, mtok * 128:(mtok + 1) * 128],
                                rhs=wqkv_t[:, kt, 2 * C + nv * 384:2 * C + (nv + 1) * 384],
                                start=(kt == 0), stop=(kt == FT - 1))
                        nc.vector.tensor_copy(
                            v_t[:, gtok, nv * 6:(nv + 1) * 6, 0:HD],
                            psv.rearrange("p (h d) -> p h d", h=6))

        # ---------------- Stage B: attention ----------------
        o_stack = ExitStack()
        o_pool = o_stack.enter_context(tc.tile_pool(name="o_sb", bufs=1))
        o_t = o_pool.tile([128, FT, T], BF16)
        with tc.tile_pool(name="exp_sb", bufs=6) as exp_pool, \
             tc.tile_pool(name="rb_sb", bufs=3) as rb_pool, \
             tc.tile_pool(name="ps_sc", bufs=3, space="PSUM") as ps_sc_pool, \
             tc.tile_pool(name="ps_av", bufs=2, space="PSUM") as ps_av_pool, \
             tc.tile_pool(name="ps_rb", bufs=2, space="PSUM") as ps_rb_pool:
            for b in range(BPC):
                for h in range(H):
                    fq = h // 2
                    po = (h % 2) * 64
                    for cq in range(NTQ):
                        tq0 = b * N + cq * CK
                        q_ap = q_t[po:po + 64, fq, tq0:tq0 + CK]
                        ps_av = ps_av_pool.tile([65, CK], F32, tag="av")
                        for tkt in range(TKT):
                            tk0 = b * N + tkt * 128
                            ps_sc = ps_sc_pool.tile([128, CK], F32, tag="sc")
                            nc.tensor.matmul(
                                ps_sc,
                                lhsT=k_t[po:po + 64, fq, tk0:tk0 + 128],
                                rhs=q_ap, start=True, stop=True)
                            e = exp_pool.tile([128, CK], BF16, tag="e")
                            nc.scalar.activation(e, ps_sc, act.Exp)
                            nc.tensor.matmul(
                                ps_av,
                                lhsT=v_t[:, b * TKT + tkt, h, :],
                                rhs=e,
                                start=(tkt == 0), stop=(tkt == TKT - 1))
                        r = rows.tile([1, CK], F32, tag="r")
                        nc.vector.reciprocal(r, ps_av[64:65, :])
                        bc = ps_rb_pool.tile([64, CK], F32, tag="rb")
                        nc.tensor.matmul(bc, lhsT=ones_row[:, 0:64], rhs=r,
                                         start=True, stop=True)
                        rb = rb_pool.tile([64, CK], F32, tag="rbs")
                        nc.vector.tensor_copy(rb, bc)
                        nc.vector.tensor_tensor(
                            o_t[po:po + 64, fq, tq0:tq0 + CK],
                            ps_av[0:64, :], rb, alu.mult)

        # ---------------- Stage C: proj + residual 1 ----------------
        x2_dram = ctx.enter_context(tc.tile_pool(name="x2d", bufs=NCH, space="DRAM"))
        x2_tiles = []
        with tc.tile_pool(name="wproj_sb2", bufs=1) as wp_pool2, \
             tc.tile_pool(name="xc2", bufs=8) as xc2_pool, \
             tc.tile_pool(name="x2s", bufs=8) as x2s_pool, \
             tc.tile_pool(name="ps_mm_c", bufs=3, space="PSUM") as ps_mm_c:
            # (wproj tile was scoped to stage B pool; reload cheaply)
            wproj_t2 = wp_pool2.tile([128, FT, C], BF16)
            for kt in range(FT):
                nc.sync.dma_start(wproj_t2[:, kt, :], wproj_d[kt, :, :])
            for ch in range(NCH):
                c0 = ch * CK
                x2d = x2_dram.tile([128, FT, CK], F32)
                x2_tiles.append(x2d)
                for mt in range(FT):
                    ps = ps_mm_c.tile([128, CK], F32, tag="mm")
                    for kt in range(FT):
                        nc.tensor.matmul(
                            ps,
                            lhsT=wproj_t2[:, kt, mt * 128:(mt + 1) * 128],
                            rhs=o_t[:, kt, c0:c0 + CK],
                            start=(kt == 0), stop=(kt == FT - 1))
                    xc2 = xc2_pool.tile([128, CK], F32, tag="xc2")
                    nc.sync.dma_start(xc2, xT_d[mt, :, c0:c0 + CK])
                    x2s = x2s_pool.tile([128, CK], F32, tag="x2s")
                    nc.vector.scalar_tensor_tensor(
                        x2s, in0=ps, scalar=bls1[:, mt:mt + 1], in1=xc2,
                        op0=alu.add, op1=alu.add)
                    nc.sync.dma_start(x2d[:, mt, :], x2s)

        o_stack.close()
        qkv_stack.close()

        # ---------------- Stage D: MLP + residual 2 ----------------
        with tc.tile_pool(name="wfc1_sb", bufs=1) as wfc1_pool, \
             tc.tile_pool(name="wfc2_sb", bufs=1) as wfc2_pool, \
             tc.tile_pool(name="xd", bufs=2) as xd_pool, \
             tc.tile_pool(name="y2", bufs=2) as y2_pool, \
             tc.tile_pool(name="h_sb", bufs=1) as h_pool, \
             tc.tile_pool(name="outs", bufs=4) as outs_pool, \
             tc.tile_pool(name="ps_rows_d", bufs=1, space="PSUM") as ps_rows_d, \
             tc.tile_pool(name="ps_bc_d", bufs=1, space="PSUM") as ps_bc_d, \
             tc.tile_pool(name="ps_mm_d", bufs=3, space="PSUM") as ps_mm_d:
            ln_pools = {"rows": rows, "scratch": scratch,
                        "ps_rows": ps_rows_d, "ps_bc": ps_bc_d}
            wfc1_t = wfc1_pool.tile([128, FT, MLP], BF16)
            for kt in range(FT):
                nc.sync.dma_start(wfc1_t[:, kt, :], wfc1_d[kt, :, :])
            wfc2_t = wfc2_pool.tile([128, VT, C], BF16)
            for kt in range(VT):
                nc.sync.dma_start(wfc2_t[:, kt, :], wfc2_d[kt, :, :])

            for ch in range(NCH):
                c0 = ch * CK
                x2c = xd_pool.tile([128, FT, CK], F32)
                for ft in range(FT):
                    nc.sync.dma_start(x2c[:, ft, :], x2_tiles[ch][:, ft, :])
                y2 = y2_pool.tile([128, FT, CK], BF16)
                _emit_ln(nc, ln_pools, x2c, ln2g, ln2b, FT, y2, ones_col,
                         ones_row, eps_row)

                h_t = h_pool.tile([128, VT, CK], BF16)
                for mt in range(VT):
                    ps = ps_mm_d.tile([128, CK], F32, tag="mm")
                    for kt in range(FT):
                        nc.tensor.matmul(
                            ps,
                            lhsT=wfc1_t[:, kt, mt * 128:(mt + 1) * 128],
                            rhs=y2[:, kt, :],
                            start=(kt == 0), stop=(kt == FT - 1))
                    nc.scalar.activation(h_t[:, mt, :], ps, act.Gelu,
                                         bias=bfc1[:, mt:mt + 1])
                for mt in range(FT):
                    ps = ps_mm_d.tile([128, CK], F32, tag="mm")
                    for kt in range(VT):
                        nc.tensor.matmul(
                            ps,
                            lhsT=wfc2_t[:, kt, mt * 128:(mt + 1) * 128],
                            rhs=h_t[:, kt, :],
                            start=(kt == 0), stop=(kt == VT - 1))
                    o_fin = outs_pool.tile([128, CK], F32, tag="o")
                    nc.vector.scalar_tensor_tensor(
                        o_fin, in0=ps, scalar=bls2[:, mt:mt + 1], in1=x2c[:, mt, :],
                        op0=alu.add, op1=alu.add)
                    nc.sync.dma_start(outT_d[mt, :, c0:c0 + CK], o_fin)

    _split_sync_waits(nc)
    _CACHE["nc"] = nc
    return nc


def _feat_cols(v):
    # [C] vector -> [128, C//128]; feature f = ft*128 + p lands at [p, ft]
    return np.ascontiguousarray(np.asarray(v, np.float32).reshape(-1, 128).T)


def make_in_maps(x, w_qkv, w_proj, b_proj, ln1_g, ln1_b, ln2_g, ln2_b,
                 ls1_g, ls2_g, w_fc1, b_fc1, w_fc2, b_fc2):
    x = np.asarray(x, np.float32)
    scale = HD ** -0.5
    wqkv = np.array(w_qkv, np.float32, copy=True)
    wqkv[:, :C] *= scale                      # fold q scaling into W_q
    wqkv = np.ascontiguousarray(wqkv.reshape(FT, 128, 3 * C).astype(BF16_NP))
    wproj = (np.asarray(w_proj, np.float32) * np.asarray(ls1_g, np.float32)[None, :])
    wproj = np.ascontiguousarray(wproj.reshape(FT, 128, C).astype(BF16_NP))
    wfc1 = np.ascontiguousarray(
        np.asarray(w_fc1, np.float32).reshape(FT, 128, MLP).astype(BF16_NP))
    wfc2 = (np.asarray(w_fc2, np.float32) * np.asarray(ls2_g, np.float32)[None, :])
    wfc2 = np.ascontiguousarray(wfc2.reshape(VT, 128, C).astype(BF16_NP))
    common = {
        "wqkv": wqkv, "wproj": wproj, "wfc1": wfc1, "wfc2": wfc2,
        "ln1g": _feat_cols(ln1_g), "ln1b": _feat_cols(ln1_b),
        "ln2g": _feat_cols(ln2_g), "ln2b": _feat_cols(ln2_b),
        "bls1": _feat_cols(np.asarray(b_proj, np.float32)
                           * np.asarray(ls1_g, np.float32)),
        "bls2": _feat_cols(np.asarray(b_fc2, np.float32)
                           * np.asarray(ls2_g, np.float32)),
        "bfc1": np.ascontiguousarray(
            np.asarray(b_fc1, np.float32).reshape(VT, 128).T),
    }
    in_maps = []
    for i in range(NCORES):
        xc = x[i * BPC:(i + 1) * BPC]                      # [BPC, N, C]
        xT = np.moveaxis(xc, 2, 0).reshape(C, T)           # [C, T]
        m = dict(common)
        m["xT"] = np.ascontiguousarray(xT.reshape(FT, 128, T))
        in_maps.append(m)
    return in_maps


def unpack_outputs(results):
    out = np.empty((B, N, C), np.float32)
    for i in range(NCORES):
        oT = results[i]["outT"].reshape(C, T)              # [C, T]
        out[i * BPC:(i + 1) * BPC] = oT.reshape(C, BPC, N).transpose(1, 2, 0)
    return out


def kernel(**inputs):
    nc = _build_program()
    in_maps = make_in_maps(**inputs)
    res = run_bass_kernel_spmd(nc, in_maps, list(range(NCORES)))
    return unpack_outputs(res.results)


if __name__ == "__main__":
    nc = _build_program()
    n_inst = sum(len(bb.instructions) for bb in nc.m.functions[0].blocks)
    print("program built OK, instructions:", n_inst)
